# revision 1
# baseline (speedup 1.0000x reference)
"""Trainium2 Bass kernel for nn_ModelSimplest_11596411699489.

Model: 4D conv (valid, 13^4 kernel, 1->3 ch, 18^4 -> 6^4) + bias + relu
       -> flatten (3888) -> dense (3888->2) + bias -> softmax.  B=512.

Mapping: the conv is lowered to matmuls over (z,w)-plane Toeplitz blocks.
For each output block (ox,oy) and each (kx,ky) kernel-plane offset, the
contribution of input plane (ox+kx, oy+ky) to all 108 outputs
(co,oz,ow) of the block is a [324 x 108] structured (Toeplitz) matrix
multiply, contracted over plane positions and accumulated in PSUM over
all 169 (kx,ky) offsets.  Plane rows are chunked 324 -> 128+128+68
partitions.  Adjacent oy blocks are paired into one N=512 matmul via a
strided y-pair access pattern (one PSUM bank per pair), so each core
holds 3 pair + 3 single accumulators (4.5 banks) and runs a single
fully-unrolled pass with x-row/weight-tile prefetch a full kx ahead.

Sharding (8 cores): output (ox,oy) 6x6 grid split into 4 quadrants of
3x3 blocks; batch split in half.  core = 4*h + q, q in [0,4) quadrant,
h in [0,2) batch half.  Each core computes feats for its 9 blocks /
256 samples, its partial dense logits, then an AllReduce over the 4
cores sharing a batch half + softmax (replicated).  Host concatenates
the two batch halves from cores 0 and 4.

Weights/planes are fed as fp16 (11-bit mantissa); accumulation is fp32
in PSUM.  Dense + softmax are fp32.
"""

import sys

if "/opt/trn_rl_repo" not in sys.path:
    sys.path.insert(0, "/opt/trn_rl_repo")

import os

import numpy as np

USE_BF16 = os.environ.get("KERNEL_BF16", "0") == "1"
if USE_BF16:
    import ml_dtypes
    NP16 = ml_dtypes.bfloat16
else:
    NP16 = np.float16

B, S, KS, SO, COUT = 512, 18, 13, 6, 3
PLANE = S * S            # 324
PCHUNKS = 3              # plane rows padded to 3*128
NB = B // 2              # batch per core (half)
M = COUT * SO * SO       # 108 outputs per block (co,oz,ow)
NBLK = 9                 # 3x3 blocks per quadrant
NT = KS * KS             # 169 (kx,ky) tap-plane offsets

_cache = {}


def _build_nc():
    import concourse.mybir as mybir
    import concourse.tile as tile
    from concourse import bacc

    f16 = mybir.dt.bfloat16 if USE_BF16 else mybir.dt.float16
    f32 = mybir.dt.float32

    nc = bacc.Bacc(num_devices=8)

    # layouts match the SBUF tile layouts so every DMA is contiguous
    # xp rows: 324 plane rows, chunked 128+128+68 onto partitions
    xp_d = nc.dram_tensor("xp", [15, PLANE, 15, NB], f16, kind="ExternalInput")
    wt_d = nc.dram_tensor("wt", [NT, 128, PCHUNKS, 128], f16, kind="ExternalInput")
    wd_d = nc.dram_tensor("wd", [NBLK, 128, 2], f32, kind="ExternalInput")
    cb_d = nc.dram_tensor("cb", [128, 1], f32, kind="ExternalInput")
    db_d = nc.dram_tensor("db", [128, 2], f32, kind="ExternalInput")
    out_d = nc.dram_tensor("out", [NB, 2], f32, kind="ExternalOutput")

    with tile.TileContext(nc) as tc:
        with (
            tc.tile_pool(name="xrows", bufs=1) as xpool,
            tc.tile_pool(name="wpool", bufs=1) as wpool,
            tc.tile_pool(name="feats", bufs=1) as fpool,
            tc.tile_pool(name="small", bufs=1) as spool,
            tc.tile_pool(name="psum", bufs=1, space="PSUM") as ppool,
            tc.tile_pool(name="dram", bufs=1, space="DRAM") as dpool,
        ):
            # constants
            cb_t = spool.tile([128, 1], f32, tag="cb")
            nc.sync.dma_start(out=cb_t[:], in_=cb_d[:])
            db_t = spool.tile([128, 2], f32, tag="db")
            nc.sync.dma_start(out=db_t[:], in_=db_d[:])
            wd_ts = []
            for bi in range(NBLK):
                t = spool.tile([128, 2], f32, tag=f"wd{bi}")
                nc.sync.dma_start(out=t[:], in_=wd_d[bi])
                wd_ts.append(t)

            # accumulators: per block-row i, a y-pair (j=0,1) -> [128, 2*NB]
            # (one psum bank) and a single (j=2) -> [128, NB].  4.5 banks.
            pair_acc = [
                ppool.tile([128, 2 * NB], f32, tag=f"pacc{i}", name=f"pacc{i}")
                for i in range(3)
            ]
            sing_acc = [
                ppool.tile([128, NB], f32, tag=f"sacc{i}", name=f"sacc{i}")
                for i in range(3)
            ]

            xrows = {}
            # chunk partition counts: 324 = 128 + 128 + 68
            CPART = (128, 128, 68)

            def load_xrow(X, fine=False, mid=None):
                # tile free layout (c, y, b); DMAs on three queues,
                # each contiguous in DRAM and in SBUF free space
                t = xpool.tile([128, PCHUNKS, 15, NB], f16, tag="xrow", bufs=5)
                if fine:
                    # split by y-range so early-ky matmuls unblock as soon as
                    # their slice lands (region-granular RAW deps)
                    for n, (y0, y1) in enumerate(((0, 5), (5, 10), (10, 15))):
                        nc.sync.dma_start(
                            out=t[:, 0, y0:y1, :], in_=xp_d[X, 0:128, y0:y1]
                        )
                        nc.scalar.dma_start(
                            out=t[:, 1, y0:y1, :], in_=xp_d[X, 128:256, y0:y1]
                        )
                        nc.gpsimd.dma_start(
                            out=t[:68, 2, y0:y1, :], in_=xp_d[X, 256:PLANE, y0:y1]
                        )
                        if n == 0 and mid is not None:
                            mid()  # urgent small loads right after first piece
                else:
                    nc.sync.dma_start(out=t[:, 0, :, :], in_=xp_d[X, 0:128])
                    nc.scalar.dma_start(out=t[:, 1, :, :], in_=xp_d[X, 128:256])
                    nc.gpsimd.dma_start(out=t[:68, 2, :, :], in_=xp_d[X, 256:PLANE])
                xrows[X] = t

            wengs = [nc.sync, nc.scalar, nc.gpsimd]
            wtiles = {}

            def load_wts(kx):
                lst = []
                for ky in range(KS):
                    w_t = wpool.tile(
                        [128, PCHUNKS, 128], f16, tag="w", bufs=32, name=f"w{kx}_{ky}"
                    )
                    wengs[ky % 3].dma_start(out=w_t[:], in_=wt_d[kx * KS + ky])
                    lst.append(w_t)
                wtiles[kx] = lst

            # preload: first x-row with the kx=0 weights interleaved right
            # after its first y-piece, then the other two x-rows
            load_xrow(0, fine=True, mid=lambda: load_wts(0))
            load_xrow(1, fine=True)
            load_xrow(2, fine=True)
            for kx in range(KS):
                # prefetch next kx's weights and x-row (a full kx of slack)
                if kx + 1 < KS:
                    load_wts(kx + 1)
                if kx + 3 <= 14:
                    load_xrow(kx + 3)
                wts = wtiles.pop(kx)
                for i in range(3):
                    xr = xrows[i + kx]
                    for ky in range(KS):
                        w_t = wts[ky]
                        for c in range(PCHUNKS):
                            kp = CPART[c]
                            first = kx == 0 and ky == 0 and c == 0
                            last = kx == KS - 1 and ky == KS - 1 and c == PCHUNKS - 1
                            nc.tensor.matmul(
                                pair_acc[i][:, :],
                                lhsT=w_t[:kp, c, :],
                                rhs=xr[:kp, c, ky : ky + 2, :],
                                start=first,
                                stop=last,
                            )
                            nc.tensor.matmul(
                                sing_acc[i][:, :],
                                lhsT=w_t[:kp, c, :],
                                rhs=xr[:kp, c, ky + 2, :],
                                start=first,
                                stop=last,
                            )
                del xrows[kx]

            # evac + relu + bias; feats[bi] is an AP [128, NB] per block
            feats = []
            for i in range(3):
                pf = fpool.tile([128, 2 * NB], f32, tag=f"pfeat{i}", name=f"pf{i}")
                nc.scalar.activation(
                    pf[:],
                    pair_acc[i][:],
                    mybir.ActivationFunctionType.Relu,
                    bias=cb_t[:],
                    scale=1.0,
                )
                sf = fpool.tile([128, NB], f32, tag=f"sfeat{i}", name=f"sf{i}")
                nc.scalar.activation(
                    sf[:],
                    sing_acc[i][:],
                    mybir.ActivationFunctionType.Relu,
                    bias=cb_t[:],
                    scale=1.0,
                )
                feats += [(pf, 0), (pf, NB), (sf, 0)]

            # dense partials: logits[b, cls] = sum_f feats[f, b] * wd[f, cls]
            cc_in = dpool.tile([2, 128, 2], f32, tag="ccin")
            cc_out = dpool.tile([2, 128, 2], f32, tag="ccout")
            for hh in range(2):
                dacc = ppool.tile([128, 2], f32, tag="dacc", bufs=2)
                for bi in range(NBLK):
                    ft, off = feats[bi]
                    nc.tensor.matmul(
                        dacc[:, :],
                        lhsT=ft[:, off + hh * 128 : off + (hh + 1) * 128],
                        rhs=wd_ts[bi][:],
                        start=(bi == 0),
                        stop=(bi == NBLK - 1),
                    )
                lg = spool.tile([128, 2], f32, tag=f"lg{hh}")
                nc.vector.tensor_copy(lg[:], dacc[:])
                nc.sync.dma_start(out=cc_in[hh], in_=lg[:])

            nc.gpsimd.collective_compute(
                "AllReduce",
                mybir.AluOpType.add,
                replica_groups=[[0, 1, 2, 3], [4, 5, 6, 7]],
                ins=[cc_in.opt()],
                outs=[cc_out.opt()],
            )

            for hh in range(2):
                lr = spool.tile([128, 2], f32, tag=f"lr{hh}")
                nc.sync.dma_start(out=lr[:], in_=cc_out[hh])
                lb = spool.tile([128, 2], f32, tag=f"lb{hh}")
                nc.vector.tensor_add(lb[:], lr[:], db_t[:])
                ex = spool.tile([128, 2], f32, tag=f"ex{hh}")
                nc.scalar.activation(
                    ex[:], lb[:], mybir.ActivationFunctionType.Exp
                )
                sm = spool.tile([128, 1], f32, tag=f"sm{hh}")
                nc.vector.reduce_sum(sm[:], ex[:], axis=mybir.AxisListType.X)
                rc = spool.tile([128, 1], f32, tag=f"rc{hh}")
                nc.vector.reciprocal(rc[:], sm[:])
                pr = spool.tile([128, 2], f32, tag=f"pr{hh}")
                nc.vector.tensor_scalar_mul(pr[:], ex[:], rc[:])
                nc.sync.dma_start(
                    out=out_d[hh * 128 : (hh + 1) * 128, :], in_=pr[:]
                )

    nc.finalize()
    return nc


def _build_wt(conv_w):
    """[3,1,13,13,13,13] f32 -> [169, 3, 128, 128] f16 Toeplitz plane tiles."""
    c_idx = np.arange(PCHUNKS)[:, None]
    r_idx = np.arange(128)[None, :]
    p = c_idx * 128 + r_idx                      # [3,128] plane row id
    z = p // S
    w_ = p % S
    pvalid = p < PLANE
    m = np.arange(M)
    co = m // (SO * SO)
    oz = (m % (SO * SO)) // SO
    ow = m % SO
    dz = z[..., None] - oz[None, None, :]        # [3,128,108]
    dw = w_[..., None] - ow[None, None, :]
    valid = pvalid[..., None] & (dz >= 0) & (dz < KS) & (dw >= 0) & (dw < KS)
    dzc = np.clip(dz, 0, KS - 1)
    dwc = np.clip(dw, 0, KS - 1)
    cw = conv_w[:, 0]                            # [3,13,13,13,13]
    wt = np.zeros((NT, 128, PCHUNKS, 128), NP16)
    cob = np.broadcast_to(co[None, None, :], dz.shape)
    for kx in range(KS):
        for ky in range(KS):
            vals = cw[cob, kx, ky, dzc, dwc]     # [3(c),128(p),108(m)]
            wt[kx * KS + ky, :, :, :M] = (
                np.where(valid, vals, 0.0).astype(NP16).transpose(1, 0, 2)
            )
    return wt


def _build_inputs(x, conv_w, conv_b, dense_w, dense_b):
    x6 = np.ascontiguousarray(x.reshape(B, S, S, PLANE))
    wt = _build_wt(conv_w)

    cb = np.zeros((128, 1), np.float32)
    cb[:M, 0] = conv_b[np.arange(M) // (SO * SO)]
    db = np.tile(dense_b[None, :].astype(np.float32), (128, 1))

    m = np.arange(M)
    co = m // (SO * SO)
    oz = (m % (SO * SO)) // SO
    ow = m % SO

    in_maps = []
    for core in range(8):
        q, h = core % 4, core // 4
        qx0, qy0 = 3 * (q // 2), 3 * (q % 2)
        slab = x6[h * NB : (h + 1) * NB, qx0 : qx0 + 15, qy0 : qy0 + 15, :]
        t = np.transpose(slab, (1, 2, 3, 0)).astype(NP16)  # [15,15,324,NB]
        # -> [X, p, y, b]: each chunk's DMA is contiguous
        xp = np.ascontiguousarray(np.transpose(t, (0, 2, 1, 3)))

        wd = np.zeros((NBLK, 128, 2), np.float32)
        for bi in range(NBLK):
            ox, oy = qx0 + bi // 3, qy0 + bi % 3
            f = co * (SO**4) + ox * (SO**3) + oy * (SO**2) + oz * SO + ow
            wd[bi, :M, :] = dense_w[:, f].T
        in_maps.append({"xp": xp, "wt": wt, "wd": wd, "cb": cb, "db": db})
    return in_maps


def _run(in_maps, trace=False):
    from concourse.bass_utils import run_bass_kernel_spmd

    if "nc" not in _cache:
        _cache["nc"] = _build_nc()
    return run_bass_kernel_spmd(_cache["nc"], in_maps, list(range(8)), trace=trace)


def kernel(x, conv_w, conv_b, dense_w, dense_b, _trace=False):
    x = np.asarray(x, np.float32)
    conv_w = np.asarray(conv_w, np.float32)
    conv_b = np.asarray(conv_b, np.float32)
    dense_w = np.asarray(dense_w, np.float32)
    dense_b = np.asarray(dense_b, np.float32)

    in_maps = _build_inputs(x, conv_w, conv_b, dense_w, dense_b)
    res = _run(in_maps, trace=_trace)
    out = np.concatenate([res.results[0]["out"], res.results[4]["out"]], axis=0)
    if _trace:
        return out, res
    return out



# revision 4
# speedup vs baseline: 1.5590x; 1.5590x over previous
"""Trainium2 Bass kernel for nn_ModelSimplest_11596411699489 (v2, fp8 DoubleRow).

Model: 4D conv (valid, 13^4 kernel, 1->3 ch, 18^4 -> 6^4) + bias + relu
       -> flatten (3888) -> dense (3888->2) + bias -> softmax.  B=512.

The conv is lowered to matmuls over (z,w)-plane Toeplitz blocks: for each
output block (ox,oy) and kernel-plane offset (kx,ky), the input plane
(ox+kx, oy+ky) contributes to the 108 outputs (co,oz,ow) via a [324 x 108]
Toeplitz matrix, accumulated in PSUM over the 169 (kx,ky) taps.

v2 uses fp8 (e4m3) inputs/weights with DoubleRow matmuls: each matmul
contracts 256 plane rows ([128 partitions x 2 blocks]).  Per tap the plane
is one DR-256 chunk (rows 0..255) plus a 68-row leftover; leftovers of 3
consecutive ky taps are stacked into one [102 x 2] DR chunk (x4), with the
tap-12 leftover as its own [34 x 2] chunk -> 18 weight chunks per kx, each
shared by 6 matmuls (3 block-rows x pair/single).  Adjacent oy blocks pair
into one N=512 matmul (one PSUM bank); oy=2 is a single N=256.  M is padded
108->112 to keep the DoubleRow weight AP 16-byte aligned and LDWEIGHTS
short.  x is scaled by 2^4 and conv weights by 2^12 before fp8 cast (both
would be subnormal otherwise); the 2^-16 descale folds into the evacuation
activation's scale.  Feats/dense run in bf16; softmax in fp32.

Sharding (8 cores): output (ox,oy) 6x6 grid split into 4 quadrants of 3x3
blocks; batch split in half.  core = 4*h + q.  Each core computes feats for
its 9 blocks / 256 samples, partial dense logits, then an AllReduce over
the 4 cores sharing a batch half + softmax (replicated).  Host concatenates
the two batch halves from cores 0 and 4.
"""

import sys

if "/opt/trn_rl_repo" not in sys.path:
    sys.path.insert(0, "/opt/trn_rl_repo")

import numpy as np
import ml_dtypes

F8 = ml_dtypes.float8_e4m3
BF16 = ml_dtypes.bfloat16

B, S, KS, SO, COUT = 512, 18, 13, 6, 3
PLANE = S * S            # 324
NB = B // 2              # batch per core (half)
M = COUT * SO * SO       # 108 outputs per block (co,oz,ow)
MP = 112                 # M padded for 16B-aligned DR weight AP
NBLK = 9                 # 3x3 blocks per quadrant
NSTK = 4                 # leftover stacks of 3 taps ({0-2},{3-5},{6-8},{9-11})
SX = 16.0                # x scale before fp8 cast
SW = 4096.0              # conv weight scale before fp8 cast
DESCALE = 1.0 / (SX * SW)

_cache = {}


def _build_nc():
    import concourse.mybir as mybir
    import concourse.tile as tile
    from concourse import bacc

    f8 = mybir.dt.float8e4
    bf = mybir.dt.bfloat16
    f32 = mybir.dt.float32
    DR = mybir.MatmulPerfMode.DoubleRow

    nc = bacc.Bacc(num_devices=8)

    # x main plane rows 0..255: [X, part r, blk j, y, b] = x[X, y, 128j+r, b]
    xm_d = nc.dram_tensor("xm", [15, 128, 2, 15, NB], f8, kind="ExternalInput")
    # leftover stacks: [X, 34u+t, G, slot i, s, b] = x[X, 3G+u+s, 256+34i+t, b]
    xs_d = nc.dram_tensor("xs", [15, 102, NSTK, 2, 3, NB], f8, kind="ExternalInput")
    # tap-12 leftover: [X, t, slot i, s, b] = x[X, 12+s, 256+34i+t, b]
    xt_d = nc.dram_tensor("xt", [15, 34, 2, 3, NB], f8, kind="ExternalInput")
    # weights (Toeplitz blocks, fp8, pre-scaled)
    wm_d = nc.dram_tensor("wm", [KS, 128, KS, 2, MP], f8, kind="ExternalInput")
    ws_d = nc.dram_tensor("ws", [KS, 102, NSTK, 2, MP], f8, kind="ExternalInput")
    wt_d = nc.dram_tensor("wt", [KS, 34, 2, MP], f8, kind="ExternalInput")
    cb_d = nc.dram_tensor("cb", [MP, 1], f32, kind="ExternalInput")
    wd_d = nc.dram_tensor("wd", [NBLK, MP, 2], bf, kind="ExternalInput")
    db_d = nc.dram_tensor("db", [128, 2], f32, kind="ExternalInput")
    out_d = nc.dram_tensor("out", [NB, 2], f32, kind="ExternalOutput")

    with tile.TileContext(nc) as tc:
        with (
            tc.tile_pool(name="xrows", bufs=1) as xpool,
            tc.tile_pool(name="wpool", bufs=1) as wpool,
            tc.tile_pool(name="feats", bufs=1) as fpool,
            tc.tile_pool(name="small", bufs=1) as spool,
            tc.tile_pool(name="psum", bufs=1, space="PSUM") as ppool,
            tc.tile_pool(name="dram", bufs=1, space="DRAM") as dpool,
        ):
            # constants
            cb_t = spool.tile([MP, 1], f32, tag="cb")
            nc.sync.dma_start(out=cb_t[:], in_=cb_d[:])
            db_t = spool.tile([128, 2], f32, tag="db")
            nc.sync.dma_start(out=db_t[:], in_=db_d[:])
            wd_ts = []
            for bi in range(NBLK):
                t = spool.tile([MP, 2], bf, tag=f"wd{bi}")
                nc.sync.dma_start(out=t[:], in_=wd_d[bi])
                wd_ts.append(t)

            # accumulators: per block-row i, a y-pair (oy=0,1) -> [MP, 2*NB]
            # (one psum bank) and a single (oy=2) -> [MP, NB].
            pair_acc = [
                ppool.tile([MP, 2 * NB], f32, tag=f"pacc{i}", name=f"pacc{i}")
                for i in range(3)
            ]
            sing_acc = [
                ppool.tile([MP, NB], f32, tag=f"sacc{i}", name=f"sacc{i}")
                for i in range(3)
            ]

            xmt, xst, xtt = {}, {}, {}

            def load_xrow(X, fine=False, mid=None):
                m_t = xpool.tile([128, 2, 15, NB], f8, tag="xm", bufs=5)
                s_t = xpool.tile([102, NSTK, 2, 3, NB], f8, tag="xs", bufs=5)
                t_t = xpool.tile([34, 2, 3, NB], f8, tag="xt", bufs=5)
                if fine:
                    # split by y so early-ky matmuls unblock as their slice lands
                    for n, (y0, y1) in enumerate(((0, 5), (5, 10), (10, 15))):
                        nc.sync.dma_start(
                            out=m_t[:, :, y0:y1, :], in_=xm_d[X, :, :, y0:y1]
                        )
                        if n == 0 and mid is not None:
                            mid()
                    nc.scalar.dma_start(out=s_t[:], in_=xs_d[X])
                    nc.gpsimd.dma_start(out=t_t[:], in_=xt_d[X])
                else:
                    nc.sync.dma_start(out=m_t[:], in_=xm_d[X])
                    nc.scalar.dma_start(out=s_t[:], in_=xs_d[X])
                    nc.gpsimd.dma_start(out=t_t[:], in_=xt_d[X])
                xmt[X], xst[X], xtt[X] = m_t, s_t, t_t

            wengs = [nc.sync, nc.scalar, nc.gpsimd]
            wtiles = {}

            def load_wts(kx):
                wm_t = wpool.tile([128, KS, 2, MP], f8, tag="wm", bufs=3)
                ws_t = wpool.tile([102, NSTK, 2, MP], f8, tag="ws", bufs=3)
                wt_t = wpool.tile([34, 2, MP], f8, tag="wt", bufs=3)
                wengs[kx % 3].dma_start(out=wm_t[:], in_=wm_d[kx])
                wengs[(kx + 1) % 3].dma_start(out=ws_t[:], in_=ws_d[kx])
                wengs[(kx + 2) % 3].dma_start(out=wt_t[:], in_=wt_d[kx])
                wtiles[kx] = (wm_t, ws_t, wt_t)

            # chunk schedule per kx: taps 3G..3G+2 then stack G; tap 12; tap-12
            # leftover last
            chunks = []
            for g in range(NSTK):
                chunks += [("m", 3 * g), ("m", 3 * g + 1), ("m", 3 * g + 2), ("s", g)]
            chunks += [("m", 12), ("t", 0)]

            load_xrow(0, fine=True, mid=lambda: load_wts(0))
            load_xrow(1, fine=True)
            load_xrow(2, fine=True)
            for kx in range(KS):
                if kx + 1 < KS:
                    load_wts(kx + 1)
                if kx + 3 <= 14:
                    load_xrow(kx + 3)
                wm_t, ws_t, wt_t = wtiles.pop(kx)
                for ci, (kind, idx) in enumerate(chunks):
                    first = kx == 0 and ci == 0
                    last = kx == KS - 1 and ci == len(chunks) - 1
                    if kind == "m":
                        lhsT = wm_t[:, idx, :, :]
                    elif kind == "s":
                        lhsT = ws_t[:, idx, :, :]
                    else:
                        lhsT = wt_t[:]
                    for i in range(3):
                        X = i + kx
                        if kind == "m":
                            rp = xmt[X][:, :, idx : idx + 2, :]
                            rs = xmt[X][:, :, idx + 2, :]
                        elif kind == "s":
                            rp = xst[X][:, idx, :, 0:2, :]
                            rs = xst[X][:, idx, :, 2, :]
                        else:
                            rp = xtt[X][:, :, 0:2, :]
                            rs = xtt[X][:, :, 2, :]
                        nc.tensor.matmul(
                            pair_acc[i][:, :], lhsT=lhsT, rhs=rp,
                            start=first, stop=last, perf_mode=DR,
                        )
                        nc.tensor.matmul(
                            sing_acc[i][:, :], lhsT=lhsT, rhs=rs,
                            start=first, stop=last, perf_mode=DR,
                        )
                del xmt[kx], xst[kx], xtt[kx]

            # evac + relu + bias + descale; feats in bf16 for the dense
            feats = []
            for i in range(3):
                pf = fpool.tile([MP, 2 * NB], bf, tag=f"pfeat{i}", name=f"pf{i}")
                nc.scalar.activation(
                    pf[:],
                    pair_acc[i][:],
                    mybir.ActivationFunctionType.Relu,
                    bias=cb_t[:],
                    scale=DESCALE,
                )
                sf = fpool.tile([MP, NB], bf, tag=f"sfeat{i}", name=f"sf{i}")
                nc.scalar.activation(
                    sf[:],
                    sing_acc[i][:],
                    mybir.ActivationFunctionType.Relu,
                    bias=cb_t[:],
                    scale=DESCALE,
                )
                feats += [(pf, 0), (pf, NB), (sf, 0)]

            # dense partials: logits[b, cls] = sum_f feats[f, b] * wd[f, cls]
            cc_in = dpool.tile([2, 128, 2], f32, tag="ccin")
            cc_out = dpool.tile([2, 128, 2], f32, tag="ccout")
            for hh in range(2):
                dacc = ppool.tile([128, 2], f32, tag="dacc", bufs=2)
                for bi in range(NBLK):
                    ft, off = feats[bi]
                    nc.tensor.matmul(
                        dacc[:, :],
                        lhsT=ft[:, off + hh * 128 : off + (hh + 1) * 128],
                        rhs=wd_ts[bi][:],
                        start=(bi == 0),
                        stop=(bi == NBLK - 1),
                    )
                lg = spool.tile([128, 2], f32, tag=f"lg{hh}")
                nc.vector.tensor_copy(lg[:], dacc[:])
                nc.sync.dma_start(out=cc_in[hh], in_=lg[:])

            nc.gpsimd.collective_compute(
                "AllReduce",
                mybir.AluOpType.add,
                replica_groups=[[0, 1, 2, 3], [4, 5, 6, 7]],
                ins=[cc_in.opt()],
                outs=[cc_out.opt()],
            )

            for hh in range(2):
                lr = spool.tile([128, 2], f32, tag=f"lr{hh}")
                nc.sync.dma_start(out=lr[:], in_=cc_out[hh])
                lb = spool.tile([128, 2], f32, tag=f"lb{hh}")
                nc.vector.tensor_add(lb[:], lr[:], db_t[:])
                ex = spool.tile([128, 2], f32, tag=f"ex{hh}")
                nc.scalar.activation(
                    ex[:], lb[:], mybir.ActivationFunctionType.Exp
                )
                sm = spool.tile([128, 1], f32, tag=f"sm{hh}")
                nc.vector.reduce_sum(sm[:], ex[:], axis=mybir.AxisListType.X)
                rc = spool.tile([128, 1], f32, tag=f"rc{hh}")
                nc.vector.reciprocal(rc[:], sm[:])
                pr = spool.tile([128, 2], f32, tag=f"pr{hh}")
                nc.vector.tensor_scalar_mul(pr[:], ex[:], rc[:])
                nc.sync.dma_start(
                    out=out_d[hh * 128 : (hh + 1) * 128, :], in_=pr[:]
                )

    nc.finalize()
    return nc


def _quant8(a):
    return np.clip(a, -240.0, 240.0).astype(F8)


def _build_w(conv_w):
    """conv_w [3,1,13,13,13,13] -> (wm, ws, wt) fp8 Toeplitz chunk tiles."""
    s = np.arange(PLANE)
    z, w_ = s // S, s % S
    m = np.arange(M)
    co = m // (SO * SO)
    oz = (m % (SO * SO)) // SO
    ow = m % SO
    dz = z[:, None] - oz[None, :]                # [324,108]
    dw = w_[:, None] - ow[None, :]
    valid = (dz >= 0) & (dz < KS) & (dw >= 0) & (dw < KS)
    dzc = np.clip(dz, 0, KS - 1)
    dwc = np.clip(dw, 0, KS - 1)
    cw = conv_w[:, 0] * SW                       # [3,13,13,13,13] scaled
    cob = np.broadcast_to(co[None, :], dz.shape)

    W = np.zeros((KS, KS, PLANE, MP), np.float32)
    for kx in range(KS):
        for ky in range(KS):
            vals = cw[cob, kx, ky, dzc, dwc]     # [324,108]
            W[kx, ky, :, :M] = np.where(valid, vals, 0.0)

    wm = np.zeros((KS, 128, KS, 2, MP), np.float32)
    ws = np.zeros((KS, 102, NSTK, 2, MP), np.float32)
    wt = np.zeros((KS, 34, 2, MP), np.float32)
    for ky in range(KS):
        for j in range(2):
            wm[:, :, ky, j, :] = W[:, ky, 128 * j : 128 * (j + 1), :]
    for g in range(NSTK):
        for u in range(3):
            for i in range(2):
                ws[:, 34 * u : 34 * (u + 1), g, i, :] = W[
                    :, 3 * g + u, 256 + 34 * i : 256 + 34 * (i + 1), :
                ]
    for i in range(2):
        wt[:, :, i, :] = W[:, 12, 256 + 34 * i : 256 + 34 * (i + 1), :]
    return _quant8(wm), _quant8(ws), _quant8(wt)


def _build_inputs(x, conv_w, conv_b, dense_w, dense_b):
    x6 = np.ascontiguousarray(x.reshape(B, S, S, PLANE))
    wm, ws, wt = _build_w(conv_w)

    m = np.arange(M)
    co = m // (SO * SO)
    oz = (m % (SO * SO)) // SO
    ow = m % SO

    cb = np.zeros((MP, 1), np.float32)
    cb[:M, 0] = conv_b[co]
    db = np.tile(dense_b[None, :].astype(np.float32), (128, 1))

    in_maps = []
    for core in range(8):
        q, h = core % 4, core // 4
        qx0, qy0 = 3 * (q // 2), 3 * (q % 2)
        slab = x6[h * NB : (h + 1) * NB, qx0 : qx0 + 15, qy0 : qy0 + 15, :]
        t = _quant8(np.transpose(slab, (1, 2, 3, 0)) * SX)  # [X, y, s, b] fp8
        # main: [15, 128, 2, 15, NB] = t[X, y, 128j+r, b] -> (X, r, j, y, b)
        xm = np.ascontiguousarray(
            t[:, :, :256, :]
            .reshape(15, 15, 2, 128, NB)
            .transpose(0, 3, 2, 1, 4)
        )
        xs = np.empty((15, 102, NSTK, 2, 3, NB), F8)
        for g in range(NSTK):
            for u in range(3):
                for i in range(2):
                    for s_ in range(3):
                        xs[:, 34 * u : 34 * (u + 1), g, i, s_, :] = t[
                            :, 3 * g + u + s_, 256 + 34 * i : 256 + 34 * (i + 1), :
                        ]
        xt = np.empty((15, 34, 2, 3, NB), F8)
        for i in range(2):
            for s_ in range(3):
                xt[:, :, i, s_, :] = t[
                    :, 12 + s_, 256 + 34 * i : 256 + 34 * (i + 1), :
                ]

        wd = np.zeros((NBLK, MP, 2), BF16)
        for bi in range(NBLK):
            ox, oy = qx0 + bi // 3, qy0 + bi % 3
            f = co * (SO**4) + ox * (SO**3) + oy * (SO**2) + oz * SO + ow
            wd[bi, :M, :] = dense_w[:, f].T.astype(BF16)
        in_maps.append(
            {"xm": xm, "xs": xs, "xt": xt, "wm": wm, "ws": ws, "wt": wt,
             "cb": cb, "wd": wd, "db": db}
        )
    return in_maps


def _run(in_maps, trace=False):
    from concourse.bass_utils import run_bass_kernel_spmd

    if "nc" not in _cache:
        _cache["nc"] = _build_nc()
    return run_bass_kernel_spmd(_cache["nc"], in_maps, list(range(8)), trace=trace)


def kernel(x, conv_w, conv_b, dense_w, dense_b, _trace=False):
    x = np.asarray(x, np.float32)
    conv_w = np.asarray(conv_w, np.float32)
    conv_b = np.asarray(conv_b, np.float32)
    dense_w = np.asarray(dense_w, np.float32)
    dense_b = np.asarray(dense_b, np.float32)

    in_maps = _build_inputs(x, conv_w, conv_b, dense_w, dense_b)
    res = _run(in_maps, trace=_trace)
    out = np.concatenate([res.results[0]["out"], res.results[4]["out"]], axis=0)
    if _trace:
        return out, res
    return out


# revision 14
# speedup vs baseline: 1.5792x; 1.0130x over previous
"""Trainium2 Bass kernel for nn_ModelSimplest_11596411699489 (v2, fp8 DoubleRow).

Model: 4D conv (valid, 13^4 kernel, 1->3 ch, 18^4 -> 6^4) + bias + relu
       -> flatten (3888) -> dense (3888->2) + bias -> softmax.  B=512.

The conv is lowered to matmuls over (z,w)-plane Toeplitz blocks: for each
output block (ox,oy) and kernel-plane offset (kx,ky), the input plane
(ox+kx, oy+ky) contributes to the 108 outputs (co,oz,ow) via a [324 x 108]
Toeplitz matrix, accumulated in PSUM over the 169 (kx,ky) taps.

v2 uses fp8 (e4m3) inputs/weights with DoubleRow matmuls: each matmul
contracts 256 plane rows ([128 partitions x 2 blocks]).  Per tap the plane
is one DR-256 chunk (rows 0..255) plus a 68-row leftover; leftovers of 3
consecutive ky taps are stacked into one [102 x 2] DR chunk (x4), with the
tap-12 leftover as its own [34 x 2] chunk -> 18 weight chunks per kx, each
shared by 6 matmuls (3 block-rows x pair/single).  Adjacent oy blocks pair
into one N=512 matmul (one PSUM bank); oy=2 is a single N=256.  M is padded
108->112 to keep the DoubleRow weight AP 16-byte aligned and LDWEIGHTS
short.  x is scaled by 2^4 and conv weights by 2^12 before fp8 cast (both
would be subnormal otherwise); the 2^-16 descale folds into the evacuation
activation's scale.  Feats/dense run in bf16; softmax in fp32.

Sharding (8 cores): output (ox,oy) 6x6 grid split into 4 quadrants of 3x3
blocks; batch split in half.  core = 4*h + q.  Each core computes feats for
its 9 blocks / 256 samples, partial dense logits, then an AllReduce over
the 4 cores sharing a batch half + softmax (replicated).  Host concatenates
the two batch halves from cores 0 and 4.
"""

import sys

if "/opt/trn_rl_repo" not in sys.path:
    sys.path.insert(0, "/opt/trn_rl_repo")

import numpy as np
import ml_dtypes

F8 = ml_dtypes.float8_e4m3
BF16 = ml_dtypes.bfloat16

B, S, KS, SO, COUT = 512, 18, 13, 6, 3
PLANE = S * S            # 324
NB = B // 2              # batch per core (half)
M = COUT * SO * SO       # 108 outputs per block (co,oz,ow)
MP = 112                 # M padded for 16B-aligned DR weight AP
NBLK = 9                 # 3x3 blocks per quadrant
NSTK = 4                 # leftover stacks of 3 taps ({0-2},{3-5},{6-8},{9-11})
SX = 16.0                # x scale before fp8 cast
SW = 4096.0              # conv weight scale before fp8 cast
DESCALE = 1.0 / (SX * SW)

_cache = {}


def _build_nc():
    import concourse.mybir as mybir
    import concourse.tile as tile
    from concourse import bacc

    f8 = mybir.dt.float8e4
    bf = mybir.dt.bfloat16
    f32 = mybir.dt.float32
    DR = mybir.MatmulPerfMode.DoubleRow

    nc = bacc.Bacc(num_devices=8)

    # x main plane rows 0..255: [X, part r, blk j, y, b] = x[X, y, 128j+r, b]
    xm_d = nc.dram_tensor("xm", [15, 128, 2, 15, NB], f8, kind="ExternalInput")
    # leftover stacks: [X, 34u+t, G, slot i, s, b] = x[X, 3G+u+s, 256+34i+t, b]
    xs_d = nc.dram_tensor("xs", [15, 102, NSTK, 2, 3, NB], f8, kind="ExternalInput")
    # tap-12 leftover: [X, t, slot i, s, b] = x[X, 12+s, 256+34i+t, b]
    xt_d = nc.dram_tensor("xt", [15, 34, 2, 3, NB], f8, kind="ExternalInput")
    # weights (Toeplitz blocks, fp8, pre-scaled)
    wm_d = nc.dram_tensor("wm", [KS, 128, KS, 2, MP], f8, kind="ExternalInput")
    ws_d = nc.dram_tensor("ws", [KS, 102, NSTK, 2, MP], f8, kind="ExternalInput")
    wt_d = nc.dram_tensor("wt", [KS, 34, 2, MP], f8, kind="ExternalInput")
    cb_d = nc.dram_tensor("cb", [MP, 1], f32, kind="ExternalInput")
    wd_d = nc.dram_tensor("wd", [NBLK, MP, 2], bf, kind="ExternalInput")
    db_d = nc.dram_tensor("db", [64, 2], f32, kind="ExternalInput")
    # each core outputs softmax for its ReduceScatter shard of 64 samples
    out_d = nc.dram_tensor("out", [64, 2], f32, kind="ExternalOutput")

    with tile.TileContext(nc) as tc:
        with (
            tc.tile_pool(name="xrows", bufs=1) as xpool,
            tc.tile_pool(name="wpool", bufs=1) as wpool,
            tc.tile_pool(name="feats", bufs=1) as fpool,
            tc.tile_pool(name="small", bufs=1) as spool,
            tc.tile_pool(name="psum", bufs=1, space="PSUM") as ppool,
            tc.tile_pool(name="dram", bufs=1, space="DRAM") as dpool,
        ):
            # accumulators: per block-row i, a y-pair (oy=0,1) -> [MP, 2*NB]
            # (one psum bank) and a single (oy=2) -> [MP, NB].
            pair_acc = [
                ppool.tile([MP, 2 * NB], f32, tag=f"pacc{i}", name=f"pacc{i}")
                for i in range(3)
            ]
            sing_acc = [
                ppool.tile([MP, NB], f32, tag=f"sacc{i}", name=f"sacc{i}")
                for i in range(3)
            ]

            xmt, xst, xtt = {}, {}, {}

            def alloc_xrow(X):
                m_t = xpool.tile([128, 2, 15, NB], f8, tag="xm", bufs=5)
                s_t = xpool.tile([102, NSTK, 2, 3, NB], f8, tag="xs", bufs=5)
                t_t = xpool.tile([34, 2, 3, NB], f8, tag="xt", bufs=5)
                xmt[X], xst[X], xtt[X] = m_t, s_t, t_t
                return m_t, s_t, t_t

            def load_xrow(X):
                m_t, s_t, t_t = alloc_xrow(X)
                nc.sync.dma_start(out=m_t[:], in_=xm_d[X])
                nc.scalar.dma_start(out=s_t[:], in_=xs_d[X])
                nc.gpsimd.dma_start(out=t_t[:], in_=xt_d[X])

            wengs = [nc.sync, nc.scalar, nc.gpsimd]
            wtiles = {}

            def load_wts(kx, first=False):
                wm_t = wpool.tile([128, KS, 2, MP], f8, tag="wm", bufs=3)
                ws_t = wpool.tile([102, NSTK, 2, MP], f8, tag="ws", bufs=3)
                wt_t = wpool.tile([34, 2, MP], f8, tag="wt", bufs=3)
                if first:
                    # kx=0: off the sync queue, which carries the first x pieces
                    nc.scalar.dma_start(out=wm_t[:], in_=wm_d[kx])
                    nc.scalar.dma_start(out=ws_t[:], in_=ws_d[kx])
                    nc.gpsimd.dma_start(out=wt_t[:], in_=wt_d[kx])
                else:
                    wengs[kx % 3].dma_start(out=wm_t[:], in_=wm_d[kx])
                    wengs[(kx + 1) % 3].dma_start(out=ws_t[:], in_=ws_d[kx])
                    wengs[(kx + 2) % 3].dma_start(out=wt_t[:], in_=wt_d[kx])
                wtiles[kx] = (wm_t, ws_t, wt_t)

            # chunk schedule per kx: taps 3G..3G+2 then stack G; tap 12; tap-12
            # leftover last
            chunks = []
            for g in range(NSTK):
                chunks += [("m", 3 * g), ("m", 3 * g + 1), ("m", 3 * g + 2), ("s", g)]
            chunks += [("m", 12), ("t", 0)]

            # startup: first weights on scalar/gpsimd; first 3 x-rows' main
            # pieces interleaved y-slice-major on sync so the kx=0 chunks
            # unblock in consumption order
            load_wts(0, first=True)
            first3 = [alloc_xrow(X) for X in range(3)]
            for y0, y1 in ((0, 5), (5, 10), (10, 15)):
                for X in range(3):
                    nc.sync.dma_start(
                        out=first3[X][0][:, :, y0:y1, :], in_=xm_d[X, :, :, y0:y1]
                    )
            for X in range(3):
                nc.scalar.dma_start(out=first3[X][1][:], in_=xs_d[X])
                nc.gpsimd.dma_start(out=first3[X][2][:], in_=xt_d[X])

            # constants after the startup-critical loads (used only at evac)
            cb_t = spool.tile([MP, 1], f32, tag="cb")
            nc.gpsimd.dma_start(out=cb_t[:], in_=cb_d[:])
            db_t = spool.tile([64, 2], f32, tag="db")
            nc.gpsimd.dma_start(out=db_t[:], in_=db_d[:])
            wd_ts = []
            for bi in range(NBLK):
                t = spool.tile([MP, 2], bf, tag=f"wd{bi}")
                nc.gpsimd.dma_start(out=t[:], in_=wd_d[bi])
                wd_ts.append(t)
            for kx in range(KS):
                if kx + 1 < KS:
                    load_wts(kx + 1)
                if kx + 3 <= 14:
                    load_xrow(kx + 3)
                wm_t, ws_t, wt_t = wtiles.pop(kx)
                for ci, (kind, idx) in enumerate(chunks):
                    first = kx == 0 and ci == 0
                    last = kx == KS - 1 and ci == len(chunks) - 1
                    if kind == "m":
                        lhsT = wm_t[:, idx, :, :]
                    elif kind == "s":
                        lhsT = ws_t[:, idx, :, :]
                    else:
                        lhsT = wt_t[:]
                    for i in range(3):
                        X = i + kx
                        if kind == "m":
                            rp = xmt[X][:, :, idx : idx + 2, :]
                            rs = xmt[X][:, :, idx + 2, :]
                        elif kind == "s":
                            rp = xst[X][:, idx, :, 0:2, :]
                            rs = xst[X][:, idx, :, 2, :]
                        else:
                            rp = xtt[X][:, :, 0:2, :]
                            rs = xtt[X][:, :, 2, :]
                        nc.tensor.matmul(
                            pair_acc[i][:, :], lhsT=lhsT, rhs=rp,
                            start=first, stop=last, perf_mode=DR,
                        )
                        nc.tensor.matmul(
                            sing_acc[i][:, :], lhsT=lhsT, rhs=rs,
                            start=first, stop=last, perf_mode=DR,
                        )
                del xmt[kx], xst[kx], xtt[kx]

            # evac + relu + bias + descale; feats in bf16 for the dense
            feats = []
            for i in range(3):
                pf = fpool.tile([MP, 2 * NB], bf, tag=f"pfeat{i}", name=f"pf{i}")
                nc.scalar.activation(
                    pf[:],
                    pair_acc[i][:],
                    mybir.ActivationFunctionType.Relu,
                    bias=cb_t[:],
                    scale=DESCALE,
                )
                sf = fpool.tile([MP, NB], bf, tag=f"sfeat{i}", name=f"sf{i}")
                nc.scalar.activation(
                    sf[:],
                    sing_acc[i][:],
                    mybir.ActivationFunctionType.Relu,
                    bias=cb_t[:],
                    scale=DESCALE,
                )
                feats += [(pf, 0), (pf, NB), (sf, 0)]

            # dense partials: logits[b, cls] = sum_f feats[f, b] * wd[f, cls]
            cc_in = dpool.tile([4, 64, 2], f32, tag="ccin")
            cc_out = dpool.tile([64, 2], f32, tag="ccout")
            for hh in range(2):
                dacc = ppool.tile([128, 2], f32, tag="dacc", bufs=2)
                for bi in range(NBLK):
                    ft, off = feats[bi]
                    nc.tensor.matmul(
                        dacc[:, :],
                        lhsT=ft[:, off + hh * 128 : off + (hh + 1) * 128],
                        rhs=wd_ts[bi][:],
                        start=(bi == 0),
                        stop=(bi == NBLK - 1),
                    )
                lg = spool.tile([128, 2], f32, tag=f"lg{hh}")
                nc.vector.tensor_copy(lg[:], dacc[:])
                nc.sync.dma_start(out=cc_in[2 * hh : 2 * hh + 2], in_=lg[:])

            # ReduceScatter: each of the 4 cores in a batch-half group gets
            # complete summed logits for its 64-sample shard (1 phase vs the
            # 2-phase AllReduce)
            nc.gpsimd.collective_compute(
                "ReduceScatter",
                mybir.AluOpType.add,
                replica_groups=[[0, 1, 2, 3], [4, 5, 6, 7]],
                ins=[cc_in.opt()],
                outs=[cc_out.opt()],
            )

            lr = spool.tile([64, 2], f32, tag="lr")
            nc.sync.dma_start(out=lr[:], in_=cc_out[:])
            lb = spool.tile([64, 2], f32, tag="lb")
            nc.vector.tensor_add(lb[:], lr[:], db_t[:])
            ex = spool.tile([64, 2], f32, tag="ex")
            nc.scalar.activation(ex[:], lb[:], mybir.ActivationFunctionType.Exp)
            sm = spool.tile([64, 1], f32, tag="sm")
            nc.vector.reduce_sum(sm[:], ex[:], axis=mybir.AxisListType.X)
            rc = spool.tile([64, 1], f32, tag="rc")
            nc.vector.reciprocal(rc[:], sm[:])
            pr = spool.tile([64, 2], f32, tag="pr")
            nc.vector.tensor_scalar_mul(pr[:], ex[:], rc[:])
            nc.sync.dma_start(out=out_d[:], in_=pr[:])

    nc.finalize()
    return nc


def _quant8(a):
    return np.clip(a, -240.0, 240.0).astype(F8)


def _build_w(conv_w):
    """conv_w [3,1,13,13,13,13] -> (wm, ws, wt) fp8 Toeplitz chunk tiles."""
    s = np.arange(PLANE)
    z, w_ = s // S, s % S
    m = np.arange(M)
    co = m // (SO * SO)
    oz = (m % (SO * SO)) // SO
    ow = m % SO
    dz = z[:, None] - oz[None, :]                # [324,108]
    dw = w_[:, None] - ow[None, :]
    valid = (dz >= 0) & (dz < KS) & (dw >= 0) & (dw < KS)
    dzc = np.clip(dz, 0, KS - 1)
    dwc = np.clip(dw, 0, KS - 1)
    cw = conv_w[:, 0] * SW                       # [3,13,13,13,13] scaled
    cob = np.broadcast_to(co[None, :], dz.shape)

    W = np.zeros((KS, KS, PLANE, MP), np.float32)
    for kx in range(KS):
        for ky in range(KS):
            vals = cw[cob, kx, ky, dzc, dwc]     # [324,108]
            W[kx, ky, :, :M] = np.where(valid, vals, 0.0)

    wm = np.zeros((KS, 128, KS, 2, MP), np.float32)
    ws = np.zeros((KS, 102, NSTK, 2, MP), np.float32)
    wt = np.zeros((KS, 34, 2, MP), np.float32)
    for ky in range(KS):
        for j in range(2):
            wm[:, :, ky, j, :] = W[:, ky, 128 * j : 128 * (j + 1), :]
    for g in range(NSTK):
        for u in range(3):
            for i in range(2):
                ws[:, 34 * u : 34 * (u + 1), g, i, :] = W[
                    :, 3 * g + u, 256 + 34 * i : 256 + 34 * (i + 1), :
                ]
    for i in range(2):
        wt[:, :, i, :] = W[:, 12, 256 + 34 * i : 256 + 34 * (i + 1), :]
    return _quant8(wm), _quant8(ws), _quant8(wt)


def _build_inputs(x, conv_w, conv_b, dense_w, dense_b):
    x6 = np.ascontiguousarray(x.reshape(B, S, S, PLANE))
    wm, ws, wt = _build_w(conv_w)

    m = np.arange(M)
    co = m // (SO * SO)
    oz = (m % (SO * SO)) // SO
    ow = m % SO

    cb = np.zeros((MP, 1), np.float32)
    cb[:M, 0] = conv_b[co]
    db = np.tile(dense_b[None, :].astype(np.float32), (64, 1))

    in_maps = []
    for core in range(8):
        q, h = core % 4, core // 4
        qx0, qy0 = 3 * (q // 2), 3 * (q % 2)
        slab = x6[h * NB : (h + 1) * NB, qx0 : qx0 + 15, qy0 : qy0 + 15, :]
        t = _quant8(np.transpose(slab, (1, 2, 3, 0)) * SX)  # [X, y, s, b] fp8
        # main: [15, 128, 2, 15, NB] = t[X, y, 128j+r, b] -> (X, r, j, y, b)
        xm = np.ascontiguousarray(
            t[:, :, :256, :]
            .reshape(15, 15, 2, 128, NB)
            .transpose(0, 3, 2, 1, 4)
        )
        xs = np.empty((15, 102, NSTK, 2, 3, NB), F8)
        for g in range(NSTK):
            for u in range(3):
                for i in range(2):
                    for s_ in range(3):
                        xs[:, 34 * u : 34 * (u + 1), g, i, s_, :] = t[
                            :, 3 * g + u + s_, 256 + 34 * i : 256 + 34 * (i + 1), :
                        ]
        xt = np.empty((15, 34, 2, 3, NB), F8)
        for i in range(2):
            for s_ in range(3):
                xt[:, :, i, s_, :] = t[
                    :, 12 + s_, 256 + 34 * i : 256 + 34 * (i + 1), :
                ]

        wd = np.zeros((NBLK, MP, 2), BF16)
        for bi in range(NBLK):
            ox, oy = qx0 + bi // 3, qy0 + bi % 3
            f = co * (SO**4) + ox * (SO**3) + oy * (SO**2) + oz * SO + ow
            wd[bi, :M, :] = dense_w[:, f].T.astype(BF16)
        in_maps.append(
            {"xm": xm, "xs": xs, "xt": xt, "wm": wm, "ws": ws, "wt": wt,
             "cb": cb, "wd": wd, "db": db}
        )
    return in_maps


def _run(in_maps, trace=False):
    from concourse.bass_utils import run_bass_kernel_spmd

    if "nc" not in _cache:
        _cache["nc"] = _build_nc()
    return run_bass_kernel_spmd(_cache["nc"], in_maps, list(range(8)), trace=trace)


def kernel(x, conv_w, conv_b, dense_w, dense_b, _trace=False):
    x = np.asarray(x, np.float32)
    conv_w = np.asarray(conv_w, np.float32)
    conv_b = np.asarray(conv_b, np.float32)
    dense_w = np.asarray(dense_w, np.float32)
    dense_b = np.asarray(dense_b, np.float32)

    in_maps = _build_inputs(x, conv_w, conv_b, dense_w, dense_b)
    res = _run(in_maps, trace=_trace)
    # core 4h+q holds the softmax for samples [256h + 64q, 256h + 64(q+1))
    out = np.concatenate([res.results[c]["out"] for c in range(8)], axis=0)
    if _trace:
        return out, res
    return out


# revision 15
# speedup vs baseline: 1.6987x; 1.0756x over previous
"""Trainium2 Bass kernel for nn_ModelSimplest_11596411699489 (v4, fp8 DoubleRow).

Model: 4D conv (valid, 13^4 kernel, 1->3 ch, 18^4 -> 6^4) + bias + relu
       -> flatten (3888) -> dense (3888->2) + bias -> softmax.  B=512.

The conv is lowered to matmuls over (z,w)-plane Toeplitz blocks: for each
output block (ox,oy) and kernel-plane offset (kx,ky), the input plane
(ox+kx, oy+ky) contributes to the 108 outputs (co,oz,ow) of the block via a
[324 x 108] Toeplitz matrix, accumulated in PSUM over the 169 (kx,ky) taps.

fp8 (e4m3) inputs/weights with DoubleRow matmuls: each matmul contracts 256
plane rows ([128 partitions x 2 slots]).  Per kx: 13 DR-256 main chunks
(plane rows 0..255 per ky) plus the 13 taps' 68-row leftovers packed
densely into 4 more DR chunks (884 rows -> 3x256 + 116, crossing tap
boundaries) = 17 weight chunks, each shared by 6 matmuls (3 block-rows x
pair/single).  Adjacent oy blocks pair into one N=512 matmul (one PSUM
bank); oy=2 is a single N=256.  M is padded 108->112 to keep the DoubleRow
weight AP 16-byte aligned.  x is scaled by 2^4 and conv weights by 2^12
before the fp8 cast (both near-subnormal otherwise); the 2^-16 descale
folds into the evacuation activation's scale.  Feats/dense in bf16;
softmax fp32.

Sharding (8 cores): output (ox,oy) 6x6 grid split into 4 quadrants of 3x3
blocks; batch split in half.  core = 4*h + q.  Each core computes feats for
its 9 blocks / 256 samples and partial dense logits; an AllReduce over the
4 cores sharing a batch half combines them, then softmax (replicated).
Host concatenates the batch halves from cores 0 and 4.  (KERNEL_COLL=rs
uses ReduceScatter + per-core 64-sample softmax instead; =ag uses
AllGather + on-device sum.)
"""

import os
import sys

if "/opt/trn_rl_repo" not in sys.path:
    sys.path.insert(0, "/opt/trn_rl_repo")

import numpy as np
import ml_dtypes

F8 = ml_dtypes.float8_e4m3
BF16 = ml_dtypes.bfloat16

B, S, KS, SO, COUT = 512, 18, 13, 6, 3
PLANE = S * S            # 324
NB = B // 2              # batch per core (half)
M = COUT * SO * SO       # 108 outputs per block (co,oz,ow)
MP = 112                 # M padded for 16B-aligned DR weight AP
NBLK = 9                 # 3x3 blocks per quadrant
LEFT = PLANE - 256       # 68 leftover rows per tap
NSTK = 4                 # leftover chunks per kx (884 rows -> 3x256+116)
SX = 16.0                # x scale before fp8 cast
SW = 4096.0              # conv weight scale before fp8 cast
DESCALE = 1.0 / (SX * SW)
COLL = os.environ.get("KERNEL_COLL", "ar")

# leftover packing: row j of the concatenated per-tap leftovers lands in
# chunk j//256, partition (j%256)//2, slot j%2
_jj = np.arange(NSTK * 256)
_valid = _jj < KS * LEFT
_taps = np.where(_valid, _jj // LEFT, 0).reshape(NSTK, 128, 2)
_rows = np.where(_valid, _jj % LEFT, 0).reshape(NSTK, 128, 2)
_mask = _valid.reshape(NSTK, 128, 2)
# chunk c may touch planes tap..tap+2; schedule it after main tap maxtap(c)
_maxtap = _taps.reshape(NSTK, -1).max(axis=1)

_cache = {}


def _build_nc():
    import concourse.mybir as mybir
    import concourse.tile as tile
    from concourse import bacc

    f8 = mybir.dt.float8e4
    bf = mybir.dt.bfloat16
    f32 = mybir.dt.float32
    DR = mybir.MatmulPerfMode.DoubleRow

    nc = bacc.Bacc(num_devices=8)

    # x main plane rows 0..255: [X, part r, slot j, y, b] = x[X, y, 128j+r, b]
    xm_d = nc.dram_tensor("xm", [15, 128, 2, 15, NB], f8, kind="ExternalInput")
    # packed leftovers: [X, r, c, slot i, s, b]
    #   = x[X, taps[c,r,i]+s, 256+rows[c,r,i], b]
    xs_d = nc.dram_tensor("xs", [15, 128, NSTK, 2, 3, NB], f8, kind="ExternalInput")
    # weights (Toeplitz blocks, fp8, pre-scaled)
    wm_d = nc.dram_tensor("wm", [KS, 128, KS, 2, MP], f8, kind="ExternalInput")
    ws_d = nc.dram_tensor("ws", [KS, 128, NSTK, 2, MP], f8, kind="ExternalInput")
    cb_d = nc.dram_tensor("cb", [MP, 1], f32, kind="ExternalInput")
    wd_d = nc.dram_tensor("wd", [NBLK, MP, 2], bf, kind="ExternalInput")
    db_d = nc.dram_tensor("db", [128, 2], f32, kind="ExternalInput")
    out_rows = 64 if COLL == "rs" else NB
    out_d = nc.dram_tensor("out", [out_rows, 2], f32, kind="ExternalOutput")

    with tile.TileContext(nc) as tc:
        with (
            tc.tile_pool(name="xrows", bufs=1) as xpool,
            tc.tile_pool(name="wpool", bufs=1) as wpool,
            tc.tile_pool(name="feats", bufs=1) as fpool,
            tc.tile_pool(name="small", bufs=1) as spool,
            tc.tile_pool(name="psum", bufs=1, space="PSUM") as ppool,
            tc.tile_pool(name="dram", bufs=1, space="DRAM") as dpool,
        ):
            # accumulators: per block-row i, a y-pair (oy=0,1) -> [MP, 2*NB]
            # (one psum bank) and a single (oy=2) -> [MP, NB].
            pair_acc = [
                ppool.tile([MP, 2 * NB], f32, tag=f"pacc{i}", name=f"pacc{i}")
                for i in range(3)
            ]
            sing_acc = [
                ppool.tile([MP, NB], f32, tag=f"sacc{i}", name=f"sacc{i}")
                for i in range(3)
            ]

            xmt, xst = {}, {}
            qengs = [nc.sync, nc.scalar, nc.gpsimd]

            def alloc_xrow(X):
                m_t = xpool.tile([128, 2, 15, NB], f8, tag="xm", bufs=5)
                s_t = xpool.tile([128, NSTK, 2, 3, NB], f8, tag="xs", bufs=5)
                xmt[X], xst[X] = m_t, s_t
                return m_t, s_t

            def load_xrow(X):
                m_t, s_t = alloc_xrow(X)
                nc.sync.dma_start(out=m_t[:], in_=xm_d[X])
                nc.scalar.dma_start(out=s_t[:], in_=xs_d[X])

            wtiles = {}

            def load_wts(kx, first=False):
                wm_t = wpool.tile([128, KS, 2, MP], f8, tag="wm", bufs=3)
                ws_t = wpool.tile([128, NSTK, 2, MP], f8, tag="ws", bufs=3)
                if first:
                    # kx=0 weights off the queues carrying the first x pieces
                    nc.gpsimd.dma_start(out=wm_t[:], in_=wm_d[kx])
                    nc.gpsimd.dma_start(out=ws_t[:], in_=ws_d[kx])
                else:
                    qengs[kx % 3].dma_start(out=wm_t[:], in_=wm_d[kx])
                    qengs[(kx + 1) % 3].dma_start(out=ws_t[:], in_=ws_d[kx])
                wtiles[kx] = (wm_t, ws_t)

            # chunk schedule per kx: main taps with leftover chunk c placed
            # once its last tap's planes are in consumption order
            chunks = []
            nxt = 0
            for ky in range(KS):
                chunks.append(("m", ky))
                while nxt < NSTK and _maxtap[nxt] <= ky:
                    chunks.append(("s", nxt))
                    nxt += 1
            assert nxt == NSTK and len(chunks) == KS + NSTK

            # startup: kx=0 weights on gpsimd; first 3 x-rows' main pieces
            # spread one queue per x-row, y-slice-major
            load_wts(0, first=True)
            first3 = [alloc_xrow(X) for X in range(3)]
            for y0, y1 in ((0, 5), (5, 10), (10, 15)):
                for X in range(3):
                    qengs[X].dma_start(
                        out=first3[X][0][:, :, y0:y1, :], in_=xm_d[X, :, :, y0:y1]
                    )
            for X in range(3):
                qengs[(X + 1) % 3].dma_start(out=first3[X][1][:], in_=xs_d[X])

            # constants after the startup-critical loads (used only at evac)
            cb_t = spool.tile([MP, 1], f32, tag="cb")
            nc.gpsimd.dma_start(out=cb_t[:], in_=cb_d[:])
            db_t = spool.tile([128, 2], f32, tag="db")
            nc.gpsimd.dma_start(out=db_t[:], in_=db_d[:])
            wd_ts = []
            for bi in range(NBLK):
                t = spool.tile([MP, 2], bf, tag=f"wd{bi}")
                nc.gpsimd.dma_start(out=t[:], in_=wd_d[bi])
                wd_ts.append(t)

            for kx in range(KS):
                if kx + 1 < KS:
                    load_wts(kx + 1)
                if kx + 3 <= 14:
                    load_xrow(kx + 3)
                wm_t, ws_t = wtiles.pop(kx)
                for ci, (kind, idx) in enumerate(chunks):
                    first = kx == 0 and ci == 0
                    last = kx == KS - 1 and ci == len(chunks) - 1
                    if kind == "m":
                        lhsT = wm_t[:, idx, :, :]
                    else:
                        lhsT = ws_t[:, idx, :, :]
                    for i in range(3):
                        X = i + kx
                        if kind == "m":
                            rp = xmt[X][:, :, idx : idx + 2, :]
                            rs = xmt[X][:, :, idx + 2, :]
                        else:
                            rp = xst[X][:, idx, :, 0:2, :]
                            rs = xst[X][:, idx, :, 2, :]
                        nc.tensor.matmul(
                            pair_acc[i][:, :], lhsT=lhsT, rhs=rp,
                            start=first, stop=last, perf_mode=DR,
                        )
                        nc.tensor.matmul(
                            sing_acc[i][:, :], lhsT=lhsT, rhs=rs,
                            start=first, stop=last, perf_mode=DR,
                        )
                del xmt[kx], xst[kx]

            # evac + relu + bias + descale; feats in bf16 for the dense
            feats = []
            for i in range(3):
                pf = fpool.tile([MP, 2 * NB], bf, tag=f"pfeat{i}", name=f"pf{i}")
                nc.scalar.activation(
                    pf[:],
                    pair_acc[i][:],
                    mybir.ActivationFunctionType.Relu,
                    bias=cb_t[:],
                    scale=DESCALE,
                )
                sf = fpool.tile([MP, NB], bf, tag=f"sfeat{i}", name=f"sf{i}")
                nc.scalar.activation(
                    sf[:],
                    sing_acc[i][:],
                    mybir.ActivationFunctionType.Relu,
                    bias=cb_t[:],
                    scale=DESCALE,
                )
                feats += [(pf, 0), (pf, NB), (sf, 0)]

            # dense partials: logits[b, cls] = sum_f feats[f, b] * wd[f, cls]
            cc_in = dpool.tile([4, 64, 2], f32, tag="ccin")
            for hh in range(2):
                dacc = ppool.tile([128, 2], f32, tag="dacc", bufs=2)
                for bi in range(NBLK):
                    ft, off = feats[bi]
                    nc.tensor.matmul(
                        dacc[:, :],
                        lhsT=ft[:, off + hh * 128 : off + (hh + 1) * 128],
                        rhs=wd_ts[bi][:],
                        start=(bi == 0),
                        stop=(bi == NBLK - 1),
                    )
                lg = spool.tile([128, 2], f32, tag=f"lg{hh}")
                nc.vector.tensor_copy(lg[:], dacc[:])
                nc.sync.dma_start(out=cc_in[2 * hh : 2 * hh + 2], in_=lg[:])

            groups = [[0, 1, 2, 3], [4, 5, 6, 7]]
            if COLL == "rs":
                cc_out = dpool.tile([64, 2], f32, tag="ccout")
                nc.gpsimd.collective_compute(
                    "ReduceScatter", mybir.AluOpType.add, replica_groups=groups,
                    ins=[cc_in.opt()], outs=[cc_out.opt()],
                )
                lr = spool.tile([64, 2], f32, tag="lr")
                nc.sync.dma_start(out=lr[:], in_=cc_out[:])
                lbs = [(lr, 64, 0)]
            elif COLL == "ag":
                cc_out = dpool.tile([4, 4, 64, 2], f32, tag="ccout")
                nc.gpsimd.collective_compute(
                    "AllGather", mybir.AluOpType.bypass, replica_groups=groups,
                    ins=[cc_in.opt()], outs=[cc_out.opt()],
                )
                lbs = []
                for hh in range(2):
                    gs = []
                    for g in range(4):
                        gt = spool.tile([128, 2], f32, tag=f"g{hh}_{g}")
                        nc.sync.dma_start(
                            out=gt[:], in_=cc_out[g, 2 * hh : 2 * hh + 2]
                        )
                        gs.append(gt)
                    a0 = spool.tile([128, 2], f32, tag=f"a0_{hh}")
                    nc.vector.tensor_add(a0[:], gs[0][:], gs[1][:])
                    a1 = spool.tile([128, 2], f32, tag=f"a1_{hh}")
                    nc.vector.tensor_add(a1[:], gs[2][:], gs[3][:])
                    a2 = spool.tile([128, 2], f32, tag=f"a2_{hh}")
                    nc.vector.tensor_add(a2[:], a0[:], a1[:])
                    lbs.append((a2, 128, hh * 128))
            else:
                cc_out = dpool.tile([4, 64, 2], f32, tag="ccout")
                nc.gpsimd.collective_compute(
                    "AllReduce", mybir.AluOpType.add, replica_groups=groups,
                    ins=[cc_in.opt()], outs=[cc_out.opt()],
                )
                lbs = []
                for hh in range(2):
                    lr = spool.tile([128, 2], f32, tag=f"lr{hh}")
                    nc.sync.dma_start(
                        out=lr[:], in_=cc_out[2 * hh : 2 * hh + 2]
                    )
                    lbs.append((lr, 128, hh * 128))

            for n, (lr, rows, o0) in enumerate(lbs):
                lb = spool.tile([rows, 2], f32, tag=f"lb{n}")
                nc.vector.tensor_add(lb[:], lr[:rows, :], db_t[:rows, :])
                ex = spool.tile([rows, 2], f32, tag=f"ex{n}")
                nc.scalar.activation(ex[:], lb[:], mybir.ActivationFunctionType.Exp)
                sm = spool.tile([rows, 1], f32, tag=f"sm{n}")
                nc.vector.reduce_sum(sm[:], ex[:], axis=mybir.AxisListType.X)
                rc = spool.tile([rows, 1], f32, tag=f"rc{n}")
                nc.vector.reciprocal(rc[:], sm[:])
                pr = spool.tile([rows, 2], f32, tag=f"pr{n}")
                nc.vector.tensor_scalar_mul(pr[:], ex[:], rc[:])
                nc.sync.dma_start(out=out_d[o0 : o0 + rows, :], in_=pr[:])

    nc.finalize()
    return nc


def _quant8(a):
    return np.clip(a, -240.0, 240.0).astype(F8)


def _build_w(conv_w):
    """conv_w [3,1,13,13,13,13] -> (wm, ws) fp8 Toeplitz chunk tiles."""
    s = np.arange(PLANE)
    z, w_ = s // S, s % S
    m = np.arange(M)
    co = m // (SO * SO)
    oz = (m % (SO * SO)) // SO
    ow = m % SO
    dz = z[:, None] - oz[None, :]                # [324,108]
    dw = w_[:, None] - ow[None, :]
    valid = (dz >= 0) & (dz < KS) & (dw >= 0) & (dw < KS)
    dzc = np.clip(dz, 0, KS - 1)
    dwc = np.clip(dw, 0, KS - 1)
    cw = conv_w[:, 0] * SW                       # [3,13,13,13,13] scaled
    cob = np.broadcast_to(co[None, :], dz.shape)

    W = np.zeros((KS, KS, PLANE, MP), np.float32)
    for kx in range(KS):
        for ky in range(KS):
            vals = cw[cob, kx, ky, dzc, dwc]     # [324,108]
            W[kx, ky, :, :M] = np.where(valid, vals, 0.0)

    wm = np.zeros((KS, 128, KS, 2, MP), np.float32)
    for ky in range(KS):
        for j in range(2):
            wm[:, :, ky, j, :] = W[:, ky, 128 * j : 128 * (j + 1), :]
    # packed leftovers: ws[kx, r, c, i, :] = W[kx, taps[c,r,i], 256+rows[c,r,i]]
    ws = W[:, _taps, 256 + _rows, :] * _mask[None, :, :, :, None]
    ws = np.ascontiguousarray(ws.transpose(0, 2, 1, 3, 4))  # [KS,128,NSTK,2,MP]
    return _quant8(wm), _quant8(ws)


def _build_inputs(x, conv_w, conv_b, dense_w, dense_b):
    x6 = np.ascontiguousarray(x.reshape(B, S, S, PLANE))
    wm, ws = _build_w(conv_w)

    m = np.arange(M)
    co = m // (SO * SO)
    oz = (m % (SO * SO)) // SO
    ow = m % SO

    cb = np.zeros((MP, 1), np.float32)
    cb[:M, 0] = conv_b[co]
    db = np.tile(dense_b[None, :].astype(np.float32), (128, 1))

    in_maps = []
    for core in range(8):
        q, h = core % 4, core // 4
        qx0, qy0 = 3 * (q // 2), 3 * (q % 2)
        slab = x6[h * NB : (h + 1) * NB, qx0 : qx0 + 15, qy0 : qy0 + 15, :]
        t = _quant8(np.transpose(slab, (1, 2, 3, 0)) * SX)  # [X, y, s, b] fp8
        # main: [15, 128, 2, 15, NB] = t[X, y, 128j+r, b] -> (X, r, j, y, b)
        xm = np.ascontiguousarray(
            t[:, :, :256, :]
            .reshape(15, 15, 2, 128, NB)
            .transpose(0, 3, 2, 1, 4)
        )
        xs = np.empty((15, 128, NSTK, 2, 3, NB), F8)
        for s_ in range(3):
            g = t[:, _taps + s_, 256 + _rows, :]  # [15, NSTK, 128, 2, NB]
            xs[:, :, :, :, s_, :] = g.transpose(0, 2, 1, 3, 4)

        wd = np.zeros((NBLK, MP, 2), BF16)
        for bi in range(NBLK):
            ox, oy = qx0 + bi // 3, qy0 + bi % 3
            f = co * (SO**4) + ox * (SO**3) + oy * (SO**2) + oz * SO + ow
            wd[bi, :M, :] = dense_w[:, f].T.astype(BF16)
        in_maps.append(
            {"xm": xm, "xs": xs, "wm": wm, "ws": ws, "cb": cb, "wd": wd, "db": db}
        )
    return in_maps


def _run(in_maps, trace=False):
    from concourse.bass_utils import run_bass_kernel_spmd

    if "nc" not in _cache:
        _cache["nc"] = _build_nc()
    return run_bass_kernel_spmd(_cache["nc"], in_maps, list(range(8)), trace=trace)


def kernel(x, conv_w, conv_b, dense_w, dense_b, _trace=False):
    x = np.asarray(x, np.float32)
    conv_w = np.asarray(conv_w, np.float32)
    conv_b = np.asarray(conv_b, np.float32)
    dense_w = np.asarray(dense_w, np.float32)
    dense_b = np.asarray(dense_b, np.float32)

    in_maps = _build_inputs(x, conv_w, conv_b, dense_w, dense_b)
    res = _run(in_maps, trace=_trace)
    if COLL == "rs":
        # core 4h+q holds the softmax for samples [256h + 64q, 256h + 64(q+1))
        out = np.concatenate([res.results[c]["out"] for c in range(8)], axis=0)
    else:
        out = np.concatenate(
            [res.results[0]["out"], res.results[4]["out"]], axis=0
        )
    if _trace:
        return out, res
    return out


# revision 16
# speedup vs baseline: 1.7712x; 1.0427x over previous
"""Trainium2 Bass kernel for nn_ModelSimplest_11596411699489 (v4, fp8 DoubleRow).

Model: 4D conv (valid, 13^4 kernel, 1->3 ch, 18^4 -> 6^4) + bias + relu
       -> flatten (3888) -> dense (3888->2) + bias -> softmax.  B=512.

The conv is lowered to matmuls over (z,w)-plane Toeplitz blocks: for each
output block (ox,oy) and kernel-plane offset (kx,ky), the input plane
(ox+kx, oy+ky) contributes to the 108 outputs (co,oz,ow) of the block via a
[324 x 108] Toeplitz matrix, accumulated in PSUM over the 169 (kx,ky) taps.

fp8 (e4m3) inputs/weights with DoubleRow matmuls: each matmul contracts 256
plane rows ([128 partitions x 2 slots]).  Per kx: 13 DR-256 main chunks
(plane rows 0..255 per ky) plus the 13 taps' 68-row leftovers packed
densely into 4 more DR chunks (884 rows -> 3x256 + 116, crossing tap
boundaries) = 17 weight chunks, each shared by 6 matmuls (3 block-rows x
pair/single).  Adjacent oy blocks pair into one N=512 matmul (one PSUM
bank); oy=2 is a single N=256.  M is padded 108->112 to keep the DoubleRow
weight AP 16-byte aligned.  x is scaled by 2^4 and conv weights by 2^12
before the fp8 cast (both near-subnormal otherwise); the 2^-16 descale
folds into the evacuation activation's scale.  Feats/dense in bf16;
softmax fp32.

Sharding (8 cores): output (ox,oy) 6x6 grid split into 4 quadrants of 3x3
blocks; batch split in half.  core = 4*h + q.  Each core computes feats for
its 9 blocks / 256 samples and partial dense logits; an AllReduce over the
4 cores sharing a batch half combines them, then softmax (replicated).
Host concatenates the batch halves from cores 0 and 4.  (KERNEL_COLL=rs
uses ReduceScatter + per-core 64-sample softmax instead; =ag uses
AllGather + on-device sum.)
"""

import os
import sys

if "/opt/trn_rl_repo" not in sys.path:
    sys.path.insert(0, "/opt/trn_rl_repo")

import numpy as np
import ml_dtypes

F8 = ml_dtypes.float8_e4m3
BF16 = ml_dtypes.bfloat16

B, S, KS, SO, COUT = 512, 18, 13, 6, 3
PLANE = S * S            # 324
NB = B // 2              # batch per core (half)
M = COUT * SO * SO       # 108 outputs per block (co,oz,ow)
MP = 112                 # M padded for 16B-aligned DR weight AP
NBLK = 9                 # 3x3 blocks per quadrant
LEFT = PLANE - 256       # 68 leftover rows per tap
NSTK = 4                 # leftover chunks per kx (884 rows -> 3x256+116)
SX = 16.0                # x scale before fp8 cast
SW = 4096.0              # conv weight scale before fp8 cast
DESCALE = 1.0 / (SX * SW)
COLL = os.environ.get("KERNEL_COLL", "ar")

# leftover packing: row j of the concatenated per-tap leftovers lands in
# chunk j//256, partition (j%256)//2, slot j%2
_jj = np.arange(NSTK * 256)
_valid = _jj < KS * LEFT
_taps = np.where(_valid, _jj // LEFT, 0).reshape(NSTK, 128, 2)
_rows = np.where(_valid, _jj % LEFT, 0).reshape(NSTK, 128, 2)
_mask = _valid.reshape(NSTK, 128, 2)
# chunk c may touch planes tap..tap+2; schedule it after main tap maxtap(c)
_maxtap = _taps.reshape(NSTK, -1).max(axis=1)

_cache = {}


def _build_nc():
    import concourse.mybir as mybir
    import concourse.tile as tile
    from concourse import bacc

    f8 = mybir.dt.float8e4
    bf = mybir.dt.bfloat16
    f32 = mybir.dt.float32
    DR = mybir.MatmulPerfMode.DoubleRow

    nc = bacc.Bacc(num_devices=8)

    # x main plane rows 0..255: [X, part r, slot j, y, b] = x[X, y, 128j+r, b]
    xm_d = nc.dram_tensor("xm", [15, 128, 2, 15, NB], f8, kind="ExternalInput")
    # packed leftovers: [X, r, c, slot i, s, b]
    #   = x[X, taps[c,r,i]+s, 256+rows[c,r,i], b]
    xs_d = nc.dram_tensor("xs", [15, 128, NSTK, 2, 3, NB], f8, kind="ExternalInput")
    # weights (Toeplitz blocks, fp8, pre-scaled)
    wm_d = nc.dram_tensor("wm", [KS, 128, KS, 2, MP], f8, kind="ExternalInput")
    ws_d = nc.dram_tensor("ws", [KS, 128, NSTK, 2, MP], f8, kind="ExternalInput")
    cb_d = nc.dram_tensor("cb", [MP, 1], f32, kind="ExternalInput")
    wd_d = nc.dram_tensor("wd", [NBLK, MP, 2], bf, kind="ExternalInput")
    db_d = nc.dram_tensor("db", [128, 2], f32, kind="ExternalInput")
    out_rows = 64 if COLL == "rs" else NB
    out_d = nc.dram_tensor("out", [out_rows, 2], f32, kind="ExternalOutput")

    with tile.TileContext(nc) as tc:
        with (
            tc.tile_pool(name="xrows", bufs=1) as xpool,
            tc.tile_pool(name="wpool", bufs=1) as wpool,
            tc.tile_pool(name="feats", bufs=1) as fpool,
            tc.tile_pool(name="small", bufs=1) as spool,
            tc.tile_pool(name="psum", bufs=1, space="PSUM") as ppool,
            tc.tile_pool(name="dram", bufs=1, space="DRAM") as dpool,
        ):
            # accumulators: per block-row i, a y-pair (oy=0,1) -> [MP, 2*NB]
            # (one psum bank) and a single (oy=2) -> [MP, NB].
            pair_acc = [
                ppool.tile([MP, 2 * NB], f32, tag=f"pacc{i}", name=f"pacc{i}")
                for i in range(3)
            ]
            sing_acc = [
                ppool.tile([MP, NB], f32, tag=f"sacc{i}", name=f"sacc{i}")
                for i in range(3)
            ]

            xmt, xst = {}, {}
            qengs = [nc.sync, nc.scalar, nc.gpsimd]

            def alloc_xrow(X):
                m_t = xpool.tile([128, 2, 15, NB], f8, tag="xm", bufs=5)
                s_t = xpool.tile([128, NSTK, 2, 3, NB], f8, tag="xs", bufs=5)
                xmt[X], xst[X] = m_t, s_t
                return m_t, s_t

            def load_xrow(X):
                m_t, s_t = alloc_xrow(X)
                nc.sync.dma_start(out=m_t[:], in_=xm_d[X])
                nc.scalar.dma_start(out=s_t[:], in_=xs_d[X])

            wtiles = {}

            def load_wts(kx, first=False):
                wm_t = wpool.tile([128, KS, 2, MP], f8, tag="wm", bufs=3)
                ws_t = wpool.tile([128, NSTK, 2, MP], f8, tag="ws", bufs=3)
                if first:
                    # kx=0 weights off the queues carrying the first x pieces
                    nc.gpsimd.dma_start(out=wm_t[:], in_=wm_d[kx])
                    nc.gpsimd.dma_start(out=ws_t[:], in_=ws_d[kx])
                else:
                    qengs[kx % 3].dma_start(out=wm_t[:], in_=wm_d[kx])
                    qengs[(kx + 1) % 3].dma_start(out=ws_t[:], in_=ws_d[kx])
                wtiles[kx] = (wm_t, ws_t)

            # chunk schedule per kx: main taps with leftover chunk c placed
            # once its last tap's planes are in consumption order
            chunks = []
            nxt = 0
            for ky in range(KS):
                chunks.append(("m", ky))
                while nxt < NSTK and _maxtap[nxt] <= ky:
                    chunks.append(("s", nxt))
                    nxt += 1
            assert nxt == NSTK and len(chunks) == KS + NSTK

            # startup: hand-scheduled so the first chunk's operands (wm ky 0:4,
            # xm y 0:3 of X=0..2) land first, one queue per x-row
            wm0 = wpool.tile([128, KS, 2, MP], f8, tag="wm", bufs=3)
            ws0 = wpool.tile([128, NSTK, 2, MP], f8, tag="ws", bufs=3)
            wtiles[0] = (wm0, ws0)
            first3 = [alloc_xrow(X) for X in range(3)]
            nc.gpsimd.dma_start(out=wm0[:, 0:4], in_=wm_d[0][:, 0:4])
            for y0, y1 in ((0, 3), (3, 8), (8, 15)):
                for X in range(3):
                    qengs[X].dma_start(
                        out=first3[X][0][:, :, y0:y1, :], in_=xm_d[X, :, :, y0:y1]
                    )
                if y0 == 3:
                    nc.gpsimd.dma_start(out=wm0[:, 4:13], in_=wm_d[0][:, 4:13])
            nc.sync.dma_start(out=first3[0][1][:], in_=xs_d[0])
            nc.scalar.dma_start(out=first3[1][1][:], in_=xs_d[1])
            nc.gpsimd.dma_start(out=ws0[:], in_=ws_d[0])
            nc.sync.dma_start(out=first3[2][1][:], in_=xs_d[2])

            # constants after the startup-critical loads (used only at evac)
            cb_t = spool.tile([MP, 1], f32, tag="cb")
            nc.gpsimd.dma_start(out=cb_t[:], in_=cb_d[:])
            db_t = spool.tile([128, 2], f32, tag="db")
            nc.gpsimd.dma_start(out=db_t[:], in_=db_d[:])
            wd_ts = []
            for bi in range(NBLK):
                t = spool.tile([MP, 2], bf, tag=f"wd{bi}")
                nc.gpsimd.dma_start(out=t[:], in_=wd_d[bi])
                wd_ts.append(t)

            for kx in range(KS):
                if kx + 1 < KS:
                    load_wts(kx + 1)
                if kx + 3 <= 14:
                    load_xrow(kx + 3)
                wm_t, ws_t = wtiles.pop(kx)
                for ci, (kind, idx) in enumerate(chunks):
                    first = kx == 0 and ci == 0
                    last = kx == KS - 1 and ci == len(chunks) - 1
                    if kind == "m":
                        lhsT = wm_t[:, idx, :, :]
                    else:
                        lhsT = ws_t[:, idx, :, :]
                    for i in range(3):
                        X = i + kx
                        if kind == "m":
                            rp = xmt[X][:, :, idx : idx + 2, :]
                            rs = xmt[X][:, :, idx + 2, :]
                        else:
                            rp = xst[X][:, idx, :, 0:2, :]
                            rs = xst[X][:, idx, :, 2, :]
                        nc.tensor.matmul(
                            pair_acc[i][:, :], lhsT=lhsT, rhs=rp,
                            start=first, stop=last, perf_mode=DR,
                        )
                        nc.tensor.matmul(
                            sing_acc[i][:, :], lhsT=lhsT, rhs=rs,
                            start=first, stop=last, perf_mode=DR,
                        )
                del xmt[kx], xst[kx]

            # evac + relu + bias + descale; feats in bf16 for the dense
            feats = []
            for i in range(3):
                pf = fpool.tile([MP, 2 * NB], bf, tag=f"pfeat{i}", name=f"pf{i}")
                nc.scalar.activation(
                    pf[:],
                    pair_acc[i][:],
                    mybir.ActivationFunctionType.Relu,
                    bias=cb_t[:],
                    scale=DESCALE,
                )
                sf = fpool.tile([MP, NB], bf, tag=f"sfeat{i}", name=f"sf{i}")
                nc.scalar.activation(
                    sf[:],
                    sing_acc[i][:],
                    mybir.ActivationFunctionType.Relu,
                    bias=cb_t[:],
                    scale=DESCALE,
                )
                feats += [(pf, 0), (pf, NB), (sf, 0)]

            # dense partials: logits[b, cls] = sum_f feats[f, b] * wd[f, cls]
            cc_in = dpool.tile([4, 64, 2], f32, tag="ccin")
            for hh in range(2):
                dacc = ppool.tile([128, 2], f32, tag="dacc", bufs=2)
                for bi in range(NBLK):
                    ft, off = feats[bi]
                    nc.tensor.matmul(
                        dacc[:, :],
                        lhsT=ft[:, off + hh * 128 : off + (hh + 1) * 128],
                        rhs=wd_ts[bi][:],
                        start=(bi == 0),
                        stop=(bi == NBLK - 1),
                    )
                lg = spool.tile([128, 2], f32, tag=f"lg{hh}")
                nc.vector.tensor_copy(lg[:], dacc[:])
                nc.sync.dma_start(out=cc_in[2 * hh : 2 * hh + 2], in_=lg[:])

            groups = [[0, 1, 2, 3], [4, 5, 6, 7]]
            if COLL == "rs":
                cc_out = dpool.tile([64, 2], f32, tag="ccout")
                nc.gpsimd.collective_compute(
                    "ReduceScatter", mybir.AluOpType.add, replica_groups=groups,
                    ins=[cc_in.opt()], outs=[cc_out.opt()],
                )
                lr = spool.tile([64, 2], f32, tag="lr")
                nc.sync.dma_start(out=lr[:], in_=cc_out[:])
                lbs = [(lr, 64, 0)]
            elif COLL == "ag":
                cc_out = dpool.tile([4, 4, 64, 2], f32, tag="ccout")
                nc.gpsimd.collective_compute(
                    "AllGather", mybir.AluOpType.bypass, replica_groups=groups,
                    ins=[cc_in.opt()], outs=[cc_out.opt()],
                )
                lbs = []
                for hh in range(2):
                    gs = []
                    for g in range(4):
                        gt = spool.tile([128, 2], f32, tag=f"g{hh}_{g}")
                        nc.sync.dma_start(
                            out=gt[:], in_=cc_out[g, 2 * hh : 2 * hh + 2]
                        )
                        gs.append(gt)
                    a0 = spool.tile([128, 2], f32, tag=f"a0_{hh}")
                    nc.vector.tensor_add(a0[:], gs[0][:], gs[1][:])
                    a1 = spool.tile([128, 2], f32, tag=f"a1_{hh}")
                    nc.vector.tensor_add(a1[:], gs[2][:], gs[3][:])
                    a2 = spool.tile([128, 2], f32, tag=f"a2_{hh}")
                    nc.vector.tensor_add(a2[:], a0[:], a1[:])
                    lbs.append((a2, 128, hh * 128))
            else:
                cc_out = dpool.tile([4, 64, 2], f32, tag="ccout")
                nc.gpsimd.collective_compute(
                    "AllReduce", mybir.AluOpType.add, replica_groups=groups,
                    ins=[cc_in.opt()], outs=[cc_out.opt()],
                )
                lbs = []
                for hh in range(2):
                    lr = spool.tile([128, 2], f32, tag=f"lr{hh}")
                    nc.sync.dma_start(
                        out=lr[:], in_=cc_out[2 * hh : 2 * hh + 2]
                    )
                    lbs.append((lr, 128, hh * 128))

            for n, (lr, rows, o0) in enumerate(lbs):
                lb = spool.tile([rows, 2], f32, tag=f"lb{n}")
                nc.vector.tensor_add(lb[:], lr[:rows, :], db_t[:rows, :])
                ex = spool.tile([rows, 2], f32, tag=f"ex{n}")
                nc.scalar.activation(ex[:], lb[:], mybir.ActivationFunctionType.Exp)
                sm = spool.tile([rows, 1], f32, tag=f"sm{n}")
                nc.vector.reduce_sum(sm[:], ex[:], axis=mybir.AxisListType.X)
                rc = spool.tile([rows, 1], f32, tag=f"rc{n}")
                nc.vector.reciprocal(rc[:], sm[:])
                pr = spool.tile([rows, 2], f32, tag=f"pr{n}")
                nc.vector.tensor_scalar_mul(pr[:], ex[:], rc[:])
                nc.sync.dma_start(out=out_d[o0 : o0 + rows, :], in_=pr[:])

    nc.finalize()
    return nc


def _quant8(a):
    return np.clip(a, -240.0, 240.0).astype(F8)


def _build_w(conv_w):
    """conv_w [3,1,13,13,13,13] -> (wm, ws) fp8 Toeplitz chunk tiles."""
    s = np.arange(PLANE)
    z, w_ = s // S, s % S
    m = np.arange(M)
    co = m // (SO * SO)
    oz = (m % (SO * SO)) // SO
    ow = m % SO
    dz = z[:, None] - oz[None, :]                # [324,108]
    dw = w_[:, None] - ow[None, :]
    valid = (dz >= 0) & (dz < KS) & (dw >= 0) & (dw < KS)
    dzc = np.clip(dz, 0, KS - 1)
    dwc = np.clip(dw, 0, KS - 1)
    cw = conv_w[:, 0] * SW                       # [3,13,13,13,13] scaled
    cob = np.broadcast_to(co[None, :], dz.shape)

    W = np.zeros((KS, KS, PLANE, MP), np.float32)
    for kx in range(KS):
        for ky in range(KS):
            vals = cw[cob, kx, ky, dzc, dwc]     # [324,108]
            W[kx, ky, :, :M] = np.where(valid, vals, 0.0)

    wm = np.zeros((KS, 128, KS, 2, MP), np.float32)
    for ky in range(KS):
        for j in range(2):
            wm[:, :, ky, j, :] = W[:, ky, 128 * j : 128 * (j + 1), :]
    # packed leftovers: ws[kx, r, c, i, :] = W[kx, taps[c,r,i], 256+rows[c,r,i]]
    ws = W[:, _taps, 256 + _rows, :] * _mask[None, :, :, :, None]
    ws = np.ascontiguousarray(ws.transpose(0, 2, 1, 3, 4))  # [KS,128,NSTK,2,MP]
    return _quant8(wm), _quant8(ws)


def _build_inputs(x, conv_w, conv_b, dense_w, dense_b):
    x6 = np.ascontiguousarray(x.reshape(B, S, S, PLANE))
    wm, ws = _build_w(conv_w)

    m = np.arange(M)
    co = m // (SO * SO)
    oz = (m % (SO * SO)) // SO
    ow = m % SO

    cb = np.zeros((MP, 1), np.float32)
    cb[:M, 0] = conv_b[co]
    db = np.tile(dense_b[None, :].astype(np.float32), (128, 1))

    in_maps = []
    for core in range(8):
        q, h = core % 4, core // 4
        qx0, qy0 = 3 * (q // 2), 3 * (q % 2)
        slab = x6[h * NB : (h + 1) * NB, qx0 : qx0 + 15, qy0 : qy0 + 15, :]
        t = _quant8(np.transpose(slab, (1, 2, 3, 0)) * SX)  # [X, y, s, b] fp8
        # main: [15, 128, 2, 15, NB] = t[X, y, 128j+r, b] -> (X, r, j, y, b)
        xm = np.ascontiguousarray(
            t[:, :, :256, :]
            .reshape(15, 15, 2, 128, NB)
            .transpose(0, 3, 2, 1, 4)
        )
        xs = np.empty((15, 128, NSTK, 2, 3, NB), F8)
        for s_ in range(3):
            g = t[:, _taps + s_, 256 + _rows, :]  # [15, NSTK, 128, 2, NB]
            xs[:, :, :, :, s_, :] = g.transpose(0, 2, 1, 3, 4)

        wd = np.zeros((NBLK, MP, 2), BF16)
        for bi in range(NBLK):
            ox, oy = qx0 + bi // 3, qy0 + bi % 3
            f = co * (SO**4) + ox * (SO**3) + oy * (SO**2) + oz * SO + ow
            wd[bi, :M, :] = dense_w[:, f].T.astype(BF16)
        in_maps.append(
            {"xm": xm, "xs": xs, "wm": wm, "ws": ws, "cb": cb, "wd": wd, "db": db}
        )
    return in_maps


def _run(in_maps, trace=False):
    from concourse.bass_utils import run_bass_kernel_spmd

    if "nc" not in _cache:
        _cache["nc"] = _build_nc()
    return run_bass_kernel_spmd(_cache["nc"], in_maps, list(range(8)), trace=trace)


def kernel(x, conv_w, conv_b, dense_w, dense_b, _trace=False):
    x = np.asarray(x, np.float32)
    conv_w = np.asarray(conv_w, np.float32)
    conv_b = np.asarray(conv_b, np.float32)
    dense_w = np.asarray(dense_w, np.float32)
    dense_b = np.asarray(dense_b, np.float32)

    in_maps = _build_inputs(x, conv_w, conv_b, dense_w, dense_b)
    res = _run(in_maps, trace=_trace)
    if COLL == "rs":
        # core 4h+q holds the softmax for samples [256h + 64q, 256h + 64(q+1))
        out = np.concatenate([res.results[c]["out"] for c in range(8)], axis=0)
    else:
        out = np.concatenate(
            [res.results[0]["out"], res.results[4]["out"]], axis=0
        )
    if _trace:
        return out, res
    return out
